# revision 29
# baseline (speedup 1.0000x reference)
"""Trainium2 Bass kernel for multi-head self-attention with RoPE.

Problem: y = MHSA(x) with
    qkv = x @ W_qkv  -> (B,S,3,H,hd) -> per-head q,k,v
    q,k = rope(q), rope(k)   (interleaved-pair rotary)
    out  = softmax(q k^T / sqrt(hd)) v
    y    = concat_heads(out) @ W_out
B=2, S=2048, E=2048, H=16, hd=128.

Sharding: 8 cores; core c handles batch b=c//4 and 4 heads h0=4*(c%4)..h0+3
(tensor-parallel over heads, data-parallel over batch). Each core computes a
partial output  O_part = attn_heads @ W_out[rows of its heads]  and the host
sums the 4 partials per batch.

Schedule (v2): three phases, engine-balanced.
  P: x and W resident in SBUF (large DMAs, 4KB lines). Per s-chunk of 512,
     per head-pair: qT/kT c-tiles via matmul (weights stationary), RoPE
     straight out of PSUM on DVE into qkrot [128, 8, S] (k tiles 0-3,
     q tiles 4-7, even dims on partitions 0-63 / odd on 64-127 via
     host-permuted W columns). v natural [S, d] via x-stationary matmuls,
     PSUM->SBUF copies on the Scalar engine (ACT idle in P).
  A: 8 blocks of (head, i-unit of 1024 queries), software-pipelined: block
     i's scores matmuls + exp (ACT) interleave slot-by-slot with block i-1's
     PV matmuls, denominator (ones-matmul), reciprocal and normalize, so the
     ACT engine (the bottleneck: 16x exp[128,1024] = 16.8us/block) never
     idles.  PSUM: scores 2x[128,1024] + PV 2x[128,512] + denom [128,512]
     = 7 banks.
  O: out-proj per 128-row i-tile: 4 PSUM banks accumulate over heads with
     stationary out2T weights amortized over 4 ec-chunks; fp16 copies +
     per-tile DMA out (output partials in fp16, host sums in fp32).
"""

import os
import math
import functools
from contextlib import ExitStack

import numpy as np

B, S, E = 2, 2048, 2048
A, H = 2048, 16
HD = A // H                     # 128
HPC = 4                         # heads per core
N_CORES = 8
THETA = 10000.0
SCALE = 1.0 / math.sqrt(HD)

LAST_RESULTS = None             # BassKernelResults of the last kernel() call


# --------------------------------------------------------------------------
# Bass program builder
# --------------------------------------------------------------------------
def build_bass():
    import concourse.bass as bass
    import concourse.mybir as mybir
    import concourse.tile as tile
    from concourse import bacc

    f32 = mybir.dt.float32
    f16 = mybir.dt.float16
    Exp = mybir.ActivationFunctionType.Exp

    s, e, hpc = S, E, HPC
    ES = e // 128               # 16 contraction subtiles
    NXT = 4                     # x/w resident tiles (es-split for DMA overlap)
    ESS = ES // NXT             # es per resident tile
    SC = 512                    # s-chunk width in projection phase
    NCH = s // SC               # 4 projection chunks
    QKT = 2 * hpc               # 8 packed c-tiles (k pairs then q pairs)
    VW = hpc * HD               # 512
    NJT = s // 128              # 16 key tiles
    IHS = 1024                  # i-unit size
    IU = s // IHS               # 2 i-units per head
    NCK = 512                   # matmul free-dim chunk (one PSUM bank fp32)
    NIC = IHS // NCK            # 2

    nc = bacc.Bacc(
        "TRN2",
        target_bir_lowering=False,
        debug=False,
        enable_asserts=False,
        num_devices=N_CORES,
    )

    xT = nc.dram_tensor("xT", (e, s), f16, kind="ExternalInput").ap()
    Wqk = nc.dram_tensor("Wqk", (e, QKT * 128), f16, kind="ExternalInput").ap()
    Wv = nc.dram_tensor("Wv", (e, VW), f16, kind="ExternalInput").ap()
    WoS = nc.dram_tensor("WoS", (VW, e), f16, kind="ExternalInput").ap()
    cosP = nc.dram_tensor("cosP", (128, s), f16, kind="ExternalInput").ap()
    sinP = nc.dram_tensor("sinP", (128, s), f16, kind="ExternalInput").ap()
    O = nc.dram_tensor("O_part", (s, e), f16, kind="ExternalOutput").ap()

    with tile.TileContext(nc) as tc, ExitStack() as octx:
        qkpool = octx.enter_context(tc.tile_pool(name="qkrot", bufs=1))
        qkrot = qkpool.tile([128, QKT, s], f16, tag="qkrot")    # k 0-3, q 4-7
        vpool = octx.enter_context(tc.tile_pool(name="vsb", bufs=1))
        v_sb = vpool.tile([128, NJT, VW], f16, tag="vsb")       # v natural

        # ====== Phase P: q/k/v projections + RoPE ==========================
        wvpool = tc.alloc_tile_pool(name="wvpool", bufs=1)
        xtpool = tc.alloc_tile_pool(name="xtpool", bufs=1)
        xpool = tc.alloc_tile_pool(name="xpool", bufs=1)
        wpool = tc.alloc_tile_pool(name="wpool", bufs=1)
        rope = tc.alloc_tile_pool(name="rope", bufs=2)
        tbl = tc.alloc_tile_pool(name="tbl", bufs=1)
        p_ps = tc.alloc_tile_pool(name="p_ps", bufs=8, space="PSUM")
        if True:
            # warm the exp table-set while everything else waits on DMA
            warm_in = tbl.tile([128, 1], f32, tag="warm_in")
            nc.vector.memset(warm_in[:], 0.0)
            warm_out = tbl.tile([128, 1], f32, tag="warm_out")
            nc.scalar.activation(warm_out[:], warm_in[:], Exp)

            # resident weights + x.  DMA order paces the pass-A matmuls:
            # k-columns of W land with each x es-block; q-columns trail.
            wq_t, xt_t = [], []
            KC = QKT // 2 * 128                 # 512 k columns
            for k in range(NXT):
                wt = wpool.tile([128, ESS, QKT * 128], f16, tag=f"wq{k}")
                wq_t.append(wt)
                xt = xpool.tile([128, ESS, s], f16, tag=f"xt{k}")
                xt_t.append(xt)

            def wq_dma(k, lo, hi):
                nc.sync.dma_start(
                    wq_t[k][:, :, lo:hi],
                    Wqk[k * ESS * 128:(k + 1) * ESS * 128, lo:hi].rearrange(
                        "(es p) c -> p es c", p=128),
                )

            wv_sb = wvpool.tile([128, ES, VW], f16, tag="wv")
            for k in range(NXT):
                if k == 0:             # split so the first matmuls start early
                    wq_dma(0, 0, KC // 2)
                    nc.sync.dma_start(
                        xt_t[0][:, 0:2, :],
                        xT[0:256, :].rearrange("(es p) s -> p es s", p=128))
                    wq_dma(0, KC // 2, KC)
                    nc.sync.dma_start(
                        xt_t[0][:, 2:4, :],
                        xT[256:512, :].rearrange("(es p) s -> p es s", p=128))
                else:
                    wq_dma(k, 0, KC)
                    nc.sync.dma_start(
                        xt_t[k][:],
                        xT[k * ESS * 128:(k + 1) * ESS * 128, :].rearrange(
                            "(es p) s -> p es s", p=128),
                    )
            nc.sync.dma_start(
                wv_sb[:], Wv.rearrange("(es p) c -> p es c", p=128))
            cos_sb = tbl.tile([128, s], f16, tag="cos")
            nc.sync.dma_start(cos_sb[:], cosP[:, :])
            sin_sb = tbl.tile([128, s], f16, tag="sin")
            nc.sync.dma_start(sin_sb[:], sinP[:, :])
            for k in range(NXT):
                wq_dma(k, KC, QKT * 128)
            xtail = xtpool.tile([128, ES, 512], f16, tag="xtail")
            nc.sync.dma_start(
                xtail[:],
                xT[:, s - 512:].rearrange("(es p) s -> p es s", p=128))

            half = HD // 2

            def qk_group(ct, c0):
                ps = p_ps.tile([128, SC], f32, name=f"qkg{ct}", tag="pp")
                for es in range(ES):
                    nc.tensor.matmul(
                        ps[:],
                        wq_t[es // ESS][:, es % ESS, ct * 128:(ct + 1) * 128],
                        xt_t[es // ESS][:, es % ESS, c0:c0 + SC],
                        start=(es == 0), stop=(es == ES - 1),
                    )
                return ps

            def v_group(st):
                ps = p_ps.tile([128, VW], f32, name="vg", tag="pp")
                for es in range(ES):
                    nc.tensor.matmul(
                        ps[:],
                        xt_t[es // ESS][:, es % ESS, st * 128:(st + 1) * 128],
                        wv_sb[:, es, :],
                        start=(es == 0), stop=(es == ES - 1),
                    )
                return ps

            def do_rope(pse, pso, ct_pair, c0):
                cs = cos_sb[:, c0:c0 + SC]
                sn = sin_sb[:, c0:c0 + SC]
                ta = rope.tile([128, SC], f32, tag="ra")
                nc.vector.tensor_mul(ta[:], pse[:], cs)
                tc2 = rope.tile([128, SC], f32, tag="rc")
                nc.vector.tensor_mul(tc2[:], pse[:], sn)
                tb = rope.tile([128, SC], f32, tag="rb")
                nc.vector.tensor_mul(tb[:], pso[:], sn)
                td = rope.tile([128, SC], f32, tag="rd")
                nc.vector.tensor_mul(td[:], pso[:], cs)
                lo, hi = ct_pair, ct_pair + 1
                # o1 = te*cs - to*sn ; o2 = te*sn + to*cs
                nc.vector.tensor_sub(
                    qkrot[0:half, lo, c0:c0 + SC], ta[0:half, :], tb[0:half, :])
                nc.vector.tensor_sub(
                    qkrot[0:half, hi, c0:c0 + SC], ta[half:128, :], tb[half:128, :])
                nc.vector.tensor_add(
                    qkrot[half:HD, lo, c0:c0 + SC], tc2[0:half, :], td[0:half, :])
                nc.vector.tensor_add(
                    qkrot[half:HD, hi, c0:c0 + SC], tc2[half:128, :], td[half:128, :])

            # --- pass A: es-outer over all k groups of chunks 0-1 so the
            # tensor engine tracks the x/W DMA arrivals -----------------------
            pa_qk = [(0, 0), (1, 0), (2, 0), (3, 0),
                     (0, 1), (1, 1), (2, 1), (3, 1)]
            pa_ps = {}
            for g in pa_qk:
                pa_ps[g] = p_ps.tile(
                    [128, SC], f32, name=f"pa_qk{g[0]}_{g[1]}", tag="pp")
            for es in range(ES):
                for (ct, ch) in pa_qk:
                    nc.tensor.matmul(
                        pa_ps[(ct, ch)][:],
                        wq_t[es // ESS][:, es % ESS, ct * 128:(ct + 1) * 128],
                        xt_t[es // ESS][:, es % ESS, ch * SC:ch * SC + SC],
                        start=(es == 0), stop=(es == ES - 1),
                    )
            for ch in (0, 1):
                do_rope(pa_ps[(0, ch)], pa_ps[(1, ch)], 0, ch * SC)
                do_rope(pa_ps[(2, ch)], pa_ps[(3, ch)], 2, ch * SC)

            # --- remaining k pairs, then v, then q pairs ---------------------
            for ch in range(2, NCH):
                for pr in (0, 1):
                    pse = qk_group(2 * pr, ch * SC)
                    pso = qk_group(2 * pr + 1, ch * SC)
                    do_rope(pse, pso, 2 * pr, ch * SC)
            for st in range(NJT - 4):
                ps = v_group(st)
                nc.scalar.copy(v_sb[:, st, :], ps[:])
            for ch in range(NCH):
                for pr in (2, 3):
                    pse = qk_group(2 * pr, ch * SC)
                    pso = qk_group(2 * pr + 1, ch * SC)
                    do_rope(pse, pso, 2 * pr, ch * SC)
        # release P-phase pools whose data is no longer needed; block 0 still
        # reads xt/wv (v fillers) and writes v_sb/qkrot.
        tbl.release()
        rope.release()
        wpool.release()
        xpool.release()
        p_ps.release()
        bsc_ps = tc.alloc_tile_pool(name="bsc_ps", bufs=2, space="PSUM")
        bvf_ps = tc.alloc_tile_pool(name="bvf_ps", bufs=2, space="PSUM")

        # ============ Phase A: attention, software-pipelined blocks ========
        expp = tc.alloc_tile_pool(name="expp", bufs=2, side="right")
        e8p = tc.alloc_tile_pool(name="e8p", bufs=2, side="right")

        # ---- block 0: scores paced against the last 4 v-projection groups --
        blocks = [(bi % hpc, bi // hpc) for bi in range(hpc * IU)]
        exptb0 = expp.tile([128, NJT, IHS], f16, tag="expt")
        e8b0 = e8p.tile([128, NJT // 2, IHS], f16, tag="e8")
        vf_ps = {}
        for slot in range(NJT):
            bps = bsc_ps.tile([128, IHS], f32, name="bps", tag="bsc")
            for icc in range(NIC):
                nc.tensor.matmul(
                    bps[:, icc * NCK:(icc + 1) * NCK],
                    qkrot[:, 0, slot * 128:(slot + 1) * 128],
                    qkrot[:, hpc, icc * NCK:(icc + 1) * NCK],
                )
            nc.scalar.activation(exptb0[:, slot, :], bps[:], Exp, scale=SCALE)
            vst = NJT - 4 + slot // 4
            if slot % 4 == 0:
                vf_ps[vst] = bvf_ps.tile([128, VW], f32, name="vf", tag="vf")
            for es in range(4 * (slot % 4), 4 * (slot % 4) + 4):
                nc.tensor.matmul(
                    vf_ps[vst][:],
                    xtail[:, es, (vst - NJT + 4) * 128:(vst - NJT + 5) * 128],
                    wv_sb[:, es, :],
                    start=(es == 0), stop=(es == ES - 1),
                )
            if slot % 4 == 3:
                nc.vector.tensor_copy(v_sb[:, vst, :], vf_ps[vst][:])
            if slot % 2 == 1:
                nc.vector.tensor_add(
                    e8b0[:, slot // 2, :], exptb0[:, slot - 1, :],
                    exptb0[:, slot, :])
        bvf_ps.release()
        bsc_ps.release()
        xtpool.release()
        wvpool.release()

        o2pool = octx.enter_context(tc.tile_pool(name="o2pool", bufs=1))
        out2T = o2pool.tile([128, hpc, s], f16, tag="out2T")
        wopool = octx.enter_context(tc.tile_pool(name="wopool", bufs=1))
        wo = wopool.tile([128, hpc, e], f16, tag="wo")
        nc.sync.dma_start(wo[:], WoS.rearrange("(h p) e -> p h e", p=128))

        with (
            tc.tile_pool(name="cpool", bufs=1) as cpool,
            tc.tile_pool(name="rbp", bufs=1) as rbp,
            tc.tile_pool(name="ofsb", bufs=2) as ofsb,
            tc.tile_pool(name="sc_ps", bufs=2, space="PSUM") as sc_ps,
            tc.tile_pool(name="un_ps", bufs=2, space="PSUM") as un_ps,
            tc.tile_pool(name="dn_ps", bufs=1, space="PSUM") as dn_ps,
            tc.tile_pool(name="of_ps", bufs=1, space="PSUM") as of_ps,
        ):
            ones16 = cpool.tile([128, 128], f16, tag="ones")
            nc.vector.memset(ones16[:], 1.0)

            st_prev = {"blk": blocks[0], "expt": exptb0, "e8": e8b0}

            NOF = 4                 # out-proj i-tiles folded into blocks 5..8
            for bi in range(1, len(blocks) + 1):
                cur = blocks[bi] if bi < len(blocks) else None
                if cur is not None:
                    h, iu = cur
                    i0 = iu * IHS
                    expt = expp.tile([128, NJT, IHS], f16, tag="expt")
                    e8 = e8p.tile([128, NJT // 2, IHS], f16, tag="e8")
                oit = bi - (len(blocks) + 1 - NOF)      # O-fill i-tile or <0
                if oit >= 0:
                    of_osb = ofsb.tile([128, e], f16, tag="ofosb")

                # fold block i-1's pair sums: 8 -> 1 tiles (DVE)
                if st_prev is not None:
                    pe8 = st_prev["e8"]
                    nt = NJT // 2
                    while nt > 1:
                        nt //= 2
                        nc.vector.tensor_add(
                            pe8[:, 0:nt, :], pe8[:, 0:nt, :], pe8[:, nt:2 * nt, :])
                    st_prev["rb"] = rbp.tile([128, IHS], f32, name="rbc",
                                             tag="rbc")
                    st_prev["rs"] = rbp.tile([128, IHS], f32, name="rscr",
                                             tag="rscr")
                    st_prev["up"] = [None] * NIC

                for slot in range(NJT):
                    if cur is not None:
                        ps = sc_ps.tile([128, IHS], f32, tag="scps")
                        for ic in range(NIC):
                            nc.tensor.matmul(
                                ps[:, ic * NCK:(ic + 1) * NCK],
                                qkrot[:, h, slot * 128:(slot + 1) * 128],
                                qkrot[:, hpc + h, i0 + ic * NCK:i0 + (ic + 1) * NCK],
                            )
                        nc.scalar.activation(
                            expt[:, slot, :], ps[:], Exp, scale=SCALE)
                        if slot % 2 == 1:
                            nc.vector.tensor_add(
                                e8[:, slot // 2, :], expt[:, slot - 1, :],
                                expt[:, slot, :])

                    if st_prev is not None:
                        ph, piu = st_prev["blk"]
                        pi0 = piu * IHS
                        pexp = st_prev["expt"]
                        ic = slot // 8
                        if slot % 8 == 0:
                            st_prev["up"][ic] = un_ps.tile(
                                [128, NCK], f32, name=f"up{ic}", tag="up")
                        up = st_prev["up"][ic]
                        for j in (2 * (slot % 8), 2 * (slot % 8) + 1):
                            nc.tensor.matmul(
                                up[:], v_sb[:, j, ph * HD:(ph + 1) * HD],
                                pexp[:, j, ic * NCK:(ic + 1) * NCK],
                                start=(j == 0), stop=(j == NJT - 1),
                                skip_group_check=True,
                            )
                        # denominator + reciprocal + normalize, per ic half
                        if slot % 8 == 5:
                            dn = dn_ps.tile([128, NCK], f32, name="dn",
                                            tag="dn")
                            st_prev["dn"] = dn
                            nc.tensor.matmul(
                                dn[:], ones16[:, :],
                                st_prev["e8"][:, 0, ic * NCK:(ic + 1) * NCK],
                                skip_group_check=True,
                            )
                            nc.vector.reciprocal_approx_accurate(
                                st_prev["rb"][:, ic * NCK:(ic + 1) * NCK],
                                dn[:],
                                st_prev["rs"][:, ic * NCK:(ic + 1) * NCK])
                        if slot % 8 == 7 and ic == 0:
                            nc.vector.tensor_mul(
                                out2T[:, ph, pi0:pi0 + NCK],
                                st_prev["up"][0][:],
                                st_prev["rb"][:, 0:NCK])

                    if oit >= 0:
                        ec, hh = slot // 4, slot % 4
                        if hh == 0:
                            ofp = of_ps.tile([128, 512], f32, name="ofp",
                                             tag="ofp")
                        nc.tensor.matmul(
                            ofp[:],
                            out2T[:, hh, oit * 128:(oit + 1) * 128],
                            wo[:, hh, ec * 512:(ec + 1) * 512],
                            start=(hh == 0), stop=(hh == hpc - 1),
                            skip_group_check=True,
                        )
                        if hh == 3:
                            nc.vector.tensor_copy(
                                of_osb[:, ec * 512:(ec + 1) * 512], ofp[:])
                            if ec == 3:
                                nc.sync.dma_start(
                                    O[oit * 128:(oit + 1) * 128, :],
                                    of_osb[:])

                if st_prev is not None:
                    ph, piu = st_prev["blk"]
                    pi0 = piu * IHS
                    nc.vector.tensor_mul(
                        out2T[:, ph, pi0 + NCK:pi0 + IHS],
                        st_prev["up"][1][:],
                        st_prev["rb"][:, NCK:IHS])

                if cur is not None:
                    st_prev = {"blk": cur, "expt": expt, "e8": e8}
                else:
                    st_prev = None
            e8p.release()
            expp.release()

        # ============ Phase O: output projection ===========================
        with (
            tc.tile_pool(name="opool", bufs=2) as opool,
            tc.tile_pool(name="o_ps", bufs=4, space="PSUM") as o_ps,
        ):
            for it in range(NOF, s // 128):
                osb = opool.tile([128, e], f16, tag="osb")
                ops = [o_ps.tile([128, 512], f32, name=f"ops{ec}",
                                 tag="ops")
                       for ec in range(e // 512)]
                for hh in range(hpc):
                    for ec in range(e // 512):
                        nc.tensor.matmul(
                            ops[ec][:],
                            out2T[:, hh, it * 128:(it + 1) * 128],
                            wo[:, hh, ec * 512:(ec + 1) * 512],
                            start=(hh == 0), stop=(hh == hpc - 1),
                            skip_group_check=True,
                        )
                        if hh == hpc - 1:
                            nc.scalar.copy(
                                osb[:, ec * 512:(ec + 1) * 512], ops[ec][:])
                            nc.sync.dma_start(
                                O[it * 128:(it + 1) * 128,
                                  ec * 512:(ec + 1) * 512],
                                osb[:, ec * 512:(ec + 1) * 512])

    nc.compile()
    return nc


# --------------------------------------------------------------------------
# Host-side prep: sharding, transposes, weight permutation, rope tables
# --------------------------------------------------------------------------
def host_prep(x, W_qkv, W_out):
    a = H * HD
    inv = 1.0 / (THETA ** (np.arange(0, HD, 2, dtype=np.float64) / HD))
    fr = np.arange(S, dtype=np.float64)[:, None] * inv[None, :]
    cos = np.cos(fr).T
    sin = np.sin(fr).T
    cosP = np.ascontiguousarray(np.concatenate([cos, cos], axis=0)).astype(np.float16)
    sinP = np.ascontiguousarray(np.concatenate([sin, sin], axis=0)).astype(np.float16)

    cores_per_batch = N_CORES // B
    in_maps = []
    for c in range(N_CORES):
        b = c // cores_per_batch
        h0 = HPC * (c % cores_per_batch)
        heads = [h0 + i for i in range(HPC)]

        xTc = np.ascontiguousarray(x[b].T).astype(np.float16)

        cols = []
        for off in (a, 0):                           # k block then q block
            for pi in range(HPC // 2):               # head pairs
                pair = heads[2 * pi:2 * pi + 2]
                for par in (0, 1):                   # even tile, odd tile
                    for hh in pair:
                        base = off + hh * HD
                        cols.extend(base + np.arange(par, HD, 2))
        Wqk = np.ascontiguousarray(W_qkv[:, np.asarray(cols)]).astype(np.float16)

        vcols = []
        for hh in heads:                             # v natural
            vcols.extend(2 * a + hh * HD + np.arange(HD))
        Wv = np.ascontiguousarray(W_qkv[:, np.asarray(vcols)]).astype(np.float16)

        rows = np.concatenate([hh * HD + np.arange(HD) for hh in heads])
        WoS = np.ascontiguousarray(W_out[rows]).astype(np.float16)

        in_maps.append({
            "xT": xTc, "Wqk": Wqk, "Wv": Wv, "WoS": WoS,
            "cosP": cosP, "sinP": sinP,
        })
    return in_maps


@functools.lru_cache(maxsize=1)
def _get_nc():
    return build_bass()


def kernel(x, W_qkv, W_out):
    global LAST_RESULTS
    from concourse import bass_utils

    x = np.ascontiguousarray(np.asarray(x, dtype=np.float32))
    W_qkv = np.ascontiguousarray(np.asarray(W_qkv, dtype=np.float32))
    W_out = np.ascontiguousarray(np.asarray(W_out, dtype=np.float32))

    nc = _get_nc()
    in_maps = host_prep(x, W_qkv, W_out)
    trace = os.environ.get("KERNEL_TRACE", "0") == "1"
    res = bass_utils.run_bass_kernel_spmd(
        nc, in_maps, core_ids=list(range(N_CORES)), trace=trace,
    )
    LAST_RESULTS = res

    cores_per_batch = N_CORES // B
    O = np.zeros((B, S, E), dtype=np.float32)
    for c in range(N_CORES):
        O[c // cores_per_batch] += res.results[c]["O_part"].astype(np.float32)
    return O


# revision 30
# speedup vs baseline: 1.0155x; 1.0155x over previous
"""Trainium2 Bass kernel for multi-head self-attention with RoPE.

Problem: y = MHSA(x) with
    qkv = x @ W_qkv  -> (B,S,3,H,hd) -> per-head q,k,v
    q,k = rope(q), rope(k)   (interleaved-pair rotary)
    out  = softmax(q k^T / sqrt(hd)) v
    y    = concat_heads(out) @ W_out
B=2, S=2048, E=2048, H=16, hd=128.

Sharding: 8 cores; core c handles batch b=c//4 and 4 heads h0=4*(c%4)..h0+3
(tensor-parallel over heads, data-parallel over batch). Each core computes a
partial output  O_part = attn_heads @ W_out[rows of its heads]  and the host
sums the 4 partials per batch.

Schedule (v2): three phases, engine-balanced.
  P: x and W resident in SBUF (large DMAs, 4KB lines). Per s-chunk of 512,
     per head-pair: qT/kT c-tiles via matmul (weights stationary), RoPE
     straight out of PSUM on DVE into qkrot [128, 8, S] (k tiles 0-3,
     q tiles 4-7, even dims on partitions 0-63 / odd on 64-127 via
     host-permuted W columns). v natural [S, d] via x-stationary matmuls,
     PSUM->SBUF copies on the Scalar engine (ACT idle in P).
  A: 8 blocks of (head, i-unit of 1024 queries), software-pipelined: block
     i's scores matmuls + exp (ACT) interleave slot-by-slot with block i-1's
     PV matmuls, denominator (ones-matmul), reciprocal and normalize, so the
     ACT engine (the bottleneck: 16x exp[128,1024] = 16.8us/block) never
     idles.  PSUM: scores 2x[128,1024] + PV 2x[128,512] + denom [128,512]
     = 7 banks.
  O: out-proj per 128-row i-tile: 4 PSUM banks accumulate over heads with
     stationary out2T weights amortized over 4 ec-chunks; fp16 copies +
     per-tile DMA out (output partials in fp16, host sums in fp32).
"""

import os
import math
import functools
from contextlib import ExitStack

import numpy as np

B, S, E = 2, 2048, 2048
A, H = 2048, 16
HD = A // H                     # 128
HPC = 4                         # heads per core
N_CORES = 8
THETA = 10000.0
SCALE = 1.0 / math.sqrt(HD)

LAST_RESULTS = None             # BassKernelResults of the last kernel() call


# --------------------------------------------------------------------------
# Bass program builder
# --------------------------------------------------------------------------
def build_bass():
    import concourse.bass as bass
    import concourse.mybir as mybir
    import concourse.tile as tile
    from concourse import bacc

    f32 = mybir.dt.float32
    f16 = mybir.dt.float16
    Exp = mybir.ActivationFunctionType.Exp

    s, e, hpc = S, E, HPC
    ES = e // 128               # 16 contraction subtiles
    NXT = 4                     # x/w resident tiles (es-split for DMA overlap)
    ESS = ES // NXT             # es per resident tile
    SC = 512                    # s-chunk width in projection phase
    NCH = s // SC               # 4 projection chunks
    QKT = 2 * hpc               # 8 packed c-tiles (k pairs then q pairs)
    VW = hpc * HD               # 512
    NJT = s // 128              # 16 key tiles
    IHS = 1024                  # i-unit size
    IU = s // IHS               # 2 i-units per head
    NCK = 512                   # matmul free-dim chunk (one PSUM bank fp32)
    NIC = IHS // NCK            # 2

    nc = bacc.Bacc(
        "TRN2",
        target_bir_lowering=False,
        debug=False,
        enable_asserts=False,
        num_devices=N_CORES,
    )

    xT = nc.dram_tensor("xT", (e, s), f16, kind="ExternalInput").ap()
    Wqk = nc.dram_tensor("Wqk", (e, QKT * 128), f16, kind="ExternalInput").ap()
    Wv = nc.dram_tensor("Wv", (e, VW), f16, kind="ExternalInput").ap()
    WoS = nc.dram_tensor("WoS", (VW, e), f16, kind="ExternalInput").ap()
    cosP = nc.dram_tensor("cosP", (128, s), f16, kind="ExternalInput").ap()
    sinP = nc.dram_tensor("sinP", (128, s), f16, kind="ExternalInput").ap()
    O = nc.dram_tensor("O_part", (s, e), f16, kind="ExternalOutput").ap()

    with tile.TileContext(nc) as tc, ExitStack() as octx:
        qkpool = octx.enter_context(tc.tile_pool(name="qkrot", bufs=1))
        qkrot = qkpool.tile([128, QKT, s], f16, tag="qkrot")    # k 0-3, q 4-7
        vpool = octx.enter_context(tc.tile_pool(name="vsb", bufs=1))
        v_sb = vpool.tile([128, NJT, VW], f16, tag="vsb")       # v natural

        # ====== Phase P: q/k/v projections + RoPE ==========================
        wvpool = tc.alloc_tile_pool(name="wvpool", bufs=1)
        xtpool = tc.alloc_tile_pool(name="xtpool", bufs=1)
        xpool = tc.alloc_tile_pool(name="xpool", bufs=1)
        wpool = tc.alloc_tile_pool(name="wpool", bufs=1)
        rope = tc.alloc_tile_pool(name="rope", bufs=2)
        tbl = tc.alloc_tile_pool(name="tbl", bufs=1)
        p_ps = tc.alloc_tile_pool(name="p_ps", bufs=8, space="PSUM")
        if True:
            # warm the exp table-set while everything else waits on DMA
            warm_in = tbl.tile([128, 1], f32, tag="warm_in")
            nc.vector.memset(warm_in[:], 0.0)
            warm_out = tbl.tile([128, 1], f32, tag="warm_out")
            nc.scalar.activation(warm_out[:], warm_in[:], Exp)

            # resident weights + x.  DMA order paces the pass-A matmuls:
            # k-columns of W land with each x es-block; q-columns trail.
            wq_t, xt_t = [], []
            KC = QKT // 2 * 128                 # 512 k columns
            for k in range(NXT):
                wt = wpool.tile([128, ESS, QKT * 128], f16, tag=f"wq{k}")
                wq_t.append(wt)
                xt = xpool.tile([128, ESS, s], f16, tag=f"xt{k}")
                xt_t.append(xt)

            def wq_dma(k, lo, hi):
                nc.sync.dma_start(
                    wq_t[k][:, :, lo:hi],
                    Wqk[k * ESS * 128:(k + 1) * ESS * 128, lo:hi].rearrange(
                        "(es p) c -> p es c", p=128),
                )

            wv_sb = wvpool.tile([128, ES, VW], f16, tag="wv")
            for k in range(NXT):
                if k == 0:             # split so the first matmuls start early
                    wq_dma(0, 0, KC // 2)
                    nc.sync.dma_start(
                        xt_t[0][:, 0:2, :],
                        xT[0:256, :].rearrange("(es p) s -> p es s", p=128))
                    wq_dma(0, KC // 2, KC)
                    nc.sync.dma_start(
                        xt_t[0][:, 2:4, :],
                        xT[256:512, :].rearrange("(es p) s -> p es s", p=128))
                else:
                    wq_dma(k, 0, KC)
                    nc.sync.dma_start(
                        xt_t[k][:],
                        xT[k * ESS * 128:(k + 1) * ESS * 128, :].rearrange(
                            "(es p) s -> p es s", p=128),
                    )
            nc.sync.dma_start(
                wv_sb[:], Wv.rearrange("(es p) c -> p es c", p=128))
            cos_sb = tbl.tile([128, s], f16, tag="cos")
            nc.sync.dma_start(cos_sb[:], cosP[:, :])
            sin_sb = tbl.tile([128, s], f16, tag="sin")
            nc.sync.dma_start(sin_sb[:], sinP[:, :])
            for k in range(NXT):
                wq_dma(k, KC, QKT * 128)
            xtail = xtpool.tile([128, ES, 512], f16, tag="xtail")
            nc.sync.dma_start(
                xtail[:],
                xT[:, s - 512:].rearrange("(es p) s -> p es s", p=128))

            half = HD // 2

            def qk_group(ct, c0):
                ps = p_ps.tile([128, SC], f32, name=f"qkg{ct}", tag="pp")
                for es in range(ES):
                    nc.tensor.matmul(
                        ps[:],
                        wq_t[es // ESS][:, es % ESS, ct * 128:(ct + 1) * 128],
                        xt_t[es // ESS][:, es % ESS, c0:c0 + SC],
                        start=(es == 0), stop=(es == ES - 1),
                    )
                return ps

            def v_group(st):
                ps = p_ps.tile([128, VW], f32, name="vg", tag="pp")
                for es in range(ES):
                    nc.tensor.matmul(
                        ps[:],
                        xt_t[es // ESS][:, es % ESS, st * 128:(st + 1) * 128],
                        wv_sb[:, es, :],
                        start=(es == 0), stop=(es == ES - 1),
                    )
                return ps

            def do_rope(pse, pso, ct_pair, c0):
                cs = cos_sb[:, c0:c0 + SC]
                sn = sin_sb[:, c0:c0 + SC]
                ta = rope.tile([128, SC], f32, tag="ra")
                nc.vector.tensor_mul(ta[:], pse[:], cs)
                tc2 = rope.tile([128, SC], f32, tag="rc")
                nc.vector.tensor_mul(tc2[:], pse[:], sn)
                tb = rope.tile([128, SC], f32, tag="rb")
                nc.vector.tensor_mul(tb[:], pso[:], sn)
                td = rope.tile([128, SC], f32, tag="rd")
                nc.vector.tensor_mul(td[:], pso[:], cs)
                lo, hi = ct_pair, ct_pair + 1
                # o1 = te*cs - to*sn ; o2 = te*sn + to*cs
                nc.vector.tensor_sub(
                    qkrot[0:half, lo, c0:c0 + SC], ta[0:half, :], tb[0:half, :])
                nc.vector.tensor_sub(
                    qkrot[0:half, hi, c0:c0 + SC], ta[half:128, :], tb[half:128, :])
                nc.vector.tensor_add(
                    qkrot[half:HD, lo, c0:c0 + SC], tc2[0:half, :], td[0:half, :])
                nc.vector.tensor_add(
                    qkrot[half:HD, hi, c0:c0 + SC], tc2[half:128, :], td[half:128, :])

            # --- pass A: es-outer over all k groups of chunks 0-1 so the
            # tensor engine tracks the x/W DMA arrivals -----------------------
            pa_qk = [(0, 0), (1, 0), (2, 0), (3, 0),
                     (0, 1), (1, 1), (2, 1), (3, 1)]
            pa_ps = {}
            for g in pa_qk:
                pa_ps[g] = p_ps.tile(
                    [128, SC], f32, name=f"pa_qk{g[0]}_{g[1]}", tag="pp")
            for es in range(ES):
                for (ct, ch) in pa_qk:
                    nc.tensor.matmul(
                        pa_ps[(ct, ch)][:],
                        wq_t[es // ESS][:, es % ESS, ct * 128:(ct + 1) * 128],
                        xt_t[es // ESS][:, es % ESS, ch * SC:ch * SC + SC],
                        start=(es == 0), stop=(es == ES - 1),
                    )
            for ch in (0, 1):
                do_rope(pa_ps[(0, ch)], pa_ps[(1, ch)], 0, ch * SC)
                do_rope(pa_ps[(2, ch)], pa_ps[(3, ch)], 2, ch * SC)

            # --- remaining k pairs, then v, then q pairs ---------------------
            for ch in range(2, NCH):
                for pr in (0, 1):
                    pse = qk_group(2 * pr, ch * SC)
                    pso = qk_group(2 * pr + 1, ch * SC)
                    do_rope(pse, pso, 2 * pr, ch * SC)
            for st in range(NJT - 4):
                ps = v_group(st)
                nc.scalar.copy(v_sb[:, st, :], ps[:])
            for ch in range(NCH):
                for pr in (2, 3):
                    pse = qk_group(2 * pr, ch * SC)
                    pso = qk_group(2 * pr + 1, ch * SC)
                    do_rope(pse, pso, 2 * pr, ch * SC)
        # release P-phase pools whose data is no longer needed; block 0 still
        # reads xt/wv (v fillers) and writes v_sb/qkrot.
        tbl.release()
        rope.release()
        wpool.release()
        xpool.release()

        # ============ Phase A: attention, software-pipelined blocks ========
        expp = tc.alloc_tile_pool(name="expp", bufs=2, side="right")
        e8p = tc.alloc_tile_pool(name="e8p", bufs=2, side="right")

        # ---- block 0: scores paced against the last 4 v-projection groups --
        blocks = [(bi % hpc, bi // hpc) for bi in range(hpc * IU)]
        exptb0 = expp.tile([128, NJT, IHS], f16, tag="expt")
        e8b0 = e8p.tile([128, NJT // 2, IHS], f16, tag="e8")
        vf_ps = {}
        for slot in range(NJT):
            for icc in range(NIC):
                bps = p_ps.tile([128, NCK], f32, name="bps", tag="pp")
                nc.tensor.matmul(
                    bps[:],
                    qkrot[:, 0, slot * 128:(slot + 1) * 128],
                    qkrot[:, hpc, icc * NCK:(icc + 1) * NCK],
                )
                nc.scalar.activation(
                    exptb0[:, slot, icc * NCK:(icc + 1) * NCK], bps[:],
                    Exp, scale=SCALE)
            vst = NJT - 4 + slot // 4
            if slot % 4 == 0:
                vf_ps[vst] = p_ps.tile([128, VW], f32, name="vf", tag="pp")
            for es in range(4 * (slot % 4), 4 * (slot % 4) + 4):
                nc.tensor.matmul(
                    vf_ps[vst][:],
                    xtail[:, es, (vst - NJT + 4) * 128:(vst - NJT + 5) * 128],
                    wv_sb[:, es, :],
                    start=(es == 0), stop=(es == ES - 1),
                )
            if slot % 4 == 3:
                nc.vector.tensor_copy(v_sb[:, vst, :], vf_ps[vst][:])
            if slot % 2 == 1:
                nc.vector.tensor_add(
                    e8b0[:, slot // 2, :], exptb0[:, slot - 1, :],
                    exptb0[:, slot, :])
        p_ps.release()
        xtpool.release()
        wvpool.release()

        o2pool = octx.enter_context(tc.tile_pool(name="o2pool", bufs=1))
        out2T = o2pool.tile([128, hpc, s], f16, tag="out2T")
        wopool = octx.enter_context(tc.tile_pool(name="wopool", bufs=1))
        wo = wopool.tile([128, hpc, e], f16, tag="wo")
        nc.sync.dma_start(wo[:], WoS.rearrange("(h p) e -> p h e", p=128))

        with (
            tc.tile_pool(name="cpool", bufs=1) as cpool,
            tc.tile_pool(name="rbp", bufs=1) as rbp,
            tc.tile_pool(name="ofsb", bufs=2) as ofsb,
            tc.tile_pool(name="sc_ps", bufs=2, space="PSUM") as sc_ps,
            tc.tile_pool(name="un_ps", bufs=2, space="PSUM") as un_ps,
            tc.tile_pool(name="dn_ps", bufs=1, space="PSUM") as dn_ps,
            tc.tile_pool(name="of_ps", bufs=1, space="PSUM") as of_ps,
        ):
            ones16 = cpool.tile([128, 128], f16, tag="ones")
            nc.vector.memset(ones16[:], 1.0)

            st_prev = {"blk": blocks[0], "expt": exptb0, "e8": e8b0}

            NOF = 4                 # out-proj i-tiles folded into blocks 5..8
            for bi in range(1, len(blocks) + 1):
                cur = blocks[bi] if bi < len(blocks) else None
                if cur is not None:
                    h, iu = cur
                    i0 = iu * IHS
                    expt = expp.tile([128, NJT, IHS], f16, tag="expt")
                    e8 = e8p.tile([128, NJT // 2, IHS], f16, tag="e8")
                oit = bi - (len(blocks) + 1 - NOF)      # O-fill i-tile or <0
                if oit >= 0:
                    of_osb = ofsb.tile([128, e], f16, tag="ofosb")

                # fold block i-1's pair sums: 8 -> 1 tiles (DVE)
                if st_prev is not None:
                    pe8 = st_prev["e8"]
                    nt = NJT // 2
                    while nt > 1:
                        nt //= 2
                        nc.vector.tensor_add(
                            pe8[:, 0:nt, :], pe8[:, 0:nt, :], pe8[:, nt:2 * nt, :])
                    st_prev["rb"] = rbp.tile([128, IHS], f32, name="rbc",
                                             tag="rbc")
                    st_prev["rs"] = rbp.tile([128, IHS], f32, name="rscr",
                                             tag="rscr")
                    st_prev["up"] = [None] * NIC

                for slot in range(NJT):
                    if cur is not None:
                        ps = sc_ps.tile([128, IHS], f32, tag="scps")
                        for ic in range(NIC):
                            nc.tensor.matmul(
                                ps[:, ic * NCK:(ic + 1) * NCK],
                                qkrot[:, h, slot * 128:(slot + 1) * 128],
                                qkrot[:, hpc + h, i0 + ic * NCK:i0 + (ic + 1) * NCK],
                            )
                        nc.scalar.activation(
                            expt[:, slot, :], ps[:], Exp, scale=SCALE)
                        if slot % 2 == 1:
                            nc.vector.tensor_add(
                                e8[:, slot // 2, :], expt[:, slot - 1, :],
                                expt[:, slot, :])

                    if st_prev is not None:
                        ph, piu = st_prev["blk"]
                        pi0 = piu * IHS
                        pexp = st_prev["expt"]
                        ic = slot // 8
                        if slot % 8 == 0:
                            st_prev["up"][ic] = un_ps.tile(
                                [128, NCK], f32, name=f"up{ic}", tag="up")
                        up = st_prev["up"][ic]
                        for j in (2 * (slot % 8), 2 * (slot % 8) + 1):
                            nc.tensor.matmul(
                                up[:], v_sb[:, j, ph * HD:(ph + 1) * HD],
                                pexp[:, j, ic * NCK:(ic + 1) * NCK],
                                start=(j == 0), stop=(j == NJT - 1),
                                skip_group_check=True,
                            )
                        # denominator + reciprocal + normalize, per ic half
                        if slot % 8 == 5:
                            dn = dn_ps.tile([128, NCK], f32, name="dn",
                                            tag="dn")
                            st_prev["dn"] = dn
                            nc.tensor.matmul(
                                dn[:], ones16[:, :],
                                st_prev["e8"][:, 0, ic * NCK:(ic + 1) * NCK],
                                skip_group_check=True,
                            )
                            nc.vector.reciprocal_approx_accurate(
                                st_prev["rb"][:, ic * NCK:(ic + 1) * NCK],
                                dn[:],
                                st_prev["rs"][:, ic * NCK:(ic + 1) * NCK])
                        if slot % 8 == 7 and ic == 0:
                            nc.vector.tensor_mul(
                                out2T[:, ph, pi0:pi0 + NCK],
                                st_prev["up"][0][:],
                                st_prev["rb"][:, 0:NCK])

                    if oit >= 0:
                        ec, hh = slot // 4, slot % 4
                        if hh == 0:
                            ofp = of_ps.tile([128, 512], f32, name="ofp",
                                             tag="ofp")
                        nc.tensor.matmul(
                            ofp[:],
                            out2T[:, hh, oit * 128:(oit + 1) * 128],
                            wo[:, hh, ec * 512:(ec + 1) * 512],
                            start=(hh == 0), stop=(hh == hpc - 1),
                            skip_group_check=True,
                        )
                        if hh == 3:
                            nc.vector.tensor_copy(
                                of_osb[:, ec * 512:(ec + 1) * 512], ofp[:])
                            if ec == 3:
                                nc.sync.dma_start(
                                    O[oit * 128:(oit + 1) * 128, :],
                                    of_osb[:])

                if st_prev is not None:
                    ph, piu = st_prev["blk"]
                    pi0 = piu * IHS
                    nc.vector.tensor_mul(
                        out2T[:, ph, pi0 + NCK:pi0 + IHS],
                        st_prev["up"][1][:],
                        st_prev["rb"][:, NCK:IHS])

                if cur is not None:
                    st_prev = {"blk": cur, "expt": expt, "e8": e8}
                else:
                    st_prev = None
            e8p.release()
            expp.release()

        # ============ Phase O: output projection ===========================
        with (
            tc.tile_pool(name="opool", bufs=2) as opool,
            tc.tile_pool(name="o_ps", bufs=4, space="PSUM") as o_ps,
        ):
            for it in range(NOF, s // 128):
                osb = opool.tile([128, e], f16, tag="osb")
                ops = [o_ps.tile([128, 512], f32, name=f"ops{ec}",
                                 tag="ops")
                       for ec in range(e // 512)]
                for hh in range(hpc):
                    for ec in range(e // 512):
                        nc.tensor.matmul(
                            ops[ec][:],
                            out2T[:, hh, it * 128:(it + 1) * 128],
                            wo[:, hh, ec * 512:(ec + 1) * 512],
                            start=(hh == 0), stop=(hh == hpc - 1),
                            skip_group_check=True,
                        )
                        if hh == hpc - 1:
                            nc.scalar.copy(
                                osb[:, ec * 512:(ec + 1) * 512], ops[ec][:])
                            nc.sync.dma_start(
                                O[it * 128:(it + 1) * 128,
                                  ec * 512:(ec + 1) * 512],
                                osb[:, ec * 512:(ec + 1) * 512])

    nc.compile()
    return nc


# --------------------------------------------------------------------------
# Host-side prep: sharding, transposes, weight permutation, rope tables
# --------------------------------------------------------------------------
def host_prep(x, W_qkv, W_out):
    a = H * HD
    inv = 1.0 / (THETA ** (np.arange(0, HD, 2, dtype=np.float64) / HD))
    fr = np.arange(S, dtype=np.float64)[:, None] * inv[None, :]
    cos = np.cos(fr).T
    sin = np.sin(fr).T
    cosP = np.ascontiguousarray(np.concatenate([cos, cos], axis=0)).astype(np.float16)
    sinP = np.ascontiguousarray(np.concatenate([sin, sin], axis=0)).astype(np.float16)

    cores_per_batch = N_CORES // B
    in_maps = []
    for c in range(N_CORES):
        b = c // cores_per_batch
        h0 = HPC * (c % cores_per_batch)
        heads = [h0 + i for i in range(HPC)]

        xTc = np.ascontiguousarray(x[b].T).astype(np.float16)

        cols = []
        for off in (a, 0):                           # k block then q block
            for pi in range(HPC // 2):               # head pairs
                pair = heads[2 * pi:2 * pi + 2]
                for par in (0, 1):                   # even tile, odd tile
                    for hh in pair:
                        base = off + hh * HD
                        cols.extend(base + np.arange(par, HD, 2))
        Wqk = np.ascontiguousarray(W_qkv[:, np.asarray(cols)]).astype(np.float16)

        vcols = []
        for hh in heads:                             # v natural
            vcols.extend(2 * a + hh * HD + np.arange(HD))
        Wv = np.ascontiguousarray(W_qkv[:, np.asarray(vcols)]).astype(np.float16)

        rows = np.concatenate([hh * HD + np.arange(HD) for hh in heads])
        WoS = np.ascontiguousarray(W_out[rows]).astype(np.float16)

        in_maps.append({
            "xT": xTc, "Wqk": Wqk, "Wv": Wv, "WoS": WoS,
            "cosP": cosP, "sinP": sinP,
        })
    return in_maps


@functools.lru_cache(maxsize=1)
def _get_nc():
    return build_bass()


def kernel(x, W_qkv, W_out):
    global LAST_RESULTS
    from concourse import bass_utils

    x = np.ascontiguousarray(np.asarray(x, dtype=np.float32))
    W_qkv = np.ascontiguousarray(np.asarray(W_qkv, dtype=np.float32))
    W_out = np.ascontiguousarray(np.asarray(W_out, dtype=np.float32))

    nc = _get_nc()
    in_maps = host_prep(x, W_qkv, W_out)
    trace = os.environ.get("KERNEL_TRACE", "0") == "1"
    res = bass_utils.run_bass_kernel_spmd(
        nc, in_maps, core_ids=list(range(N_CORES)), trace=trace,
    )
    LAST_RESULTS = res

    cores_per_batch = N_CORES // B
    O = np.zeros((B, S, E), dtype=np.float32)
    for c in range(N_CORES):
        O[c // cores_per_batch] += res.results[c]["O_part"].astype(np.float32)
    return O
